# revision 1
# baseline (speedup 1.0000x reference)
"""Trainium2 Bass kernel for nn_AffineTransformLayer (B=8, C=4, H=W=1024).

Strategy (pure data parallel, batch element b -> NeuronCore b):
  1. Host computes, bit-exactly mirroring the jax-CPU reference, the per-pixel
     gather indices (i1, i0) and the four scalar blend weights per batch.
     (The reference's XLA-CPU einsum uses FMA contractions that cannot be
     reproduced bit-exactly by two-rounding device arithmetic; a 1-ulp index
     difference flips floor() and corrupts pixels, so indices ship from host.)
  2. Device premixes the 4 bilinear corners into one image V per channel
     (V[r,c] = w00*x[r,c] + w10*x[r,c+1] + w01*x[r+1,c] + w11*x[r+1,c+1]),
     so each output pixel becomes a single gather: out[y,x] = V[i1, i0].
  3. The 2D gather runs as 1024 tiles of 32x32 output pixels. Each tile gets
     a per-partition SBUF window = a dynamically anchored 184x148 box of V
     (fetched with a register-offset DMA) plus the 4 boundary lines of V
     (for clamped pixels outside the box). A GPSIMD ap_gather resolves the
     per-pixel window indices (host-packed int16 streams).
  4. Pixels whose sources fall outside box+lines (only possible for violently
     expansive transforms) are patched on host; for the benchmark inputs this
     is a tiny remainder and usually zero.
"""

import os
from contextlib import ExitStack

import numpy as np

H = W = 1024
C = 4
B = 8
TS = 32
NT = H // TS            # 32 tiles per side
NTILES = NT * NT        # 1024
NG = 8                  # ap_gather groups (Q7 cores)
NWAVES = NTILES // NG   # 128
HWIN, WWIN = 192, 148   # box dims (HWIN 8-aligned for the V_p8 layout)
NGRP8 = HWIN // 8       # 24 row-groups per box
BOX = HWIN * WWIN       # 28416
NLINE = 4 * H           # 4096
NE = BOX + NLINE        # 32512 window elements per partition
NIDX = TS * TS          # 1024 stream indices per tile
NBLK = H // 128         # 8 premix row blocks
# Per-group partition offset of the 4-channel window quad. Offsets chosen so
# each group's box-fetch DMA straddles an SDMA-engine boundary and the 8
# groups together cover all 16 engines (engine 2k+h serves partitions
# {64h + 4k + {0..3}, 64h + 32 + 4k + {0..3}}).
DQUAD = [2, 2, 10, 10, 2, 2, 10, 10]

_cache = {}


def _build_program():
    import concourse.bass as bass
    import concourse.bacc as bacc
    import concourse.tile as tile
    from concourse import mybir

    f32 = mybir.dt.float32
    i32 = mybir.dt.int32
    i16 = mybir.dt.int16
    Alu = mybir.AluOpType

    nc = bacc.Bacc("TRN2", target_bir_lowering=False, debug=False)
    x = nc.dram_tensor("x", [C, H, W], f32, kind="ExternalInput").ap()
    wts = nc.dram_tensor("wts", [1, 4], f32, kind="ExternalInput").ap()
    anc = nc.dram_tensor("anc", [1, NTILES], i32, kind="ExternalInput").ap()
    idxs = nc.dram_tensor("idxs", [NWAVES, 128, NIDX // 16], i16, kind="ExternalInput").ap()
    out = nc.dram_tensor("out", [C, H, W], f32, kind="ExternalOutput").ap()
    v = nc.dram_tensor("v", [C, H, W], f32).ap()
    vp8 = nc.dram_tensor("vp8", [C, H * W], f32).ap()
    ln = nc.dram_tensor("ln", [C, 4 * H], f32).ap()

    with tile.TileContext(nc) as tc, ExitStack() as ctx:
        cpool = ctx.enter_context(tc.tile_pool(name="const", bufs=1))

        wt = cpool.tile([128, 4], f32)
        nc.sync.dma_start(wt[:], wts[0:1, :].partition_broadcast(128))
        anct = cpool.tile([1, NTILES], i32)
        nc.sync.dma_start(anct[:], anc[:, :])

        # ---- premix: V = 4-corner blend of x ----
        with tc.tile_pool(name="pmx", bufs=2) as pmx:
            for blk in range(NBLK):
                y0 = blk * 128
                t0 = pmx.tile([128, C * W], f32, tag="t0")
                t0v = t0[:].rearrange("p (c n) -> p c n", c=C)
                nc.sync.dma_start(t0v, x[0:C, y0 : y0 + 128, :].transpose([1, 0, 2]))
                t1 = pmx.tile([128, C * W], f32, tag="t1")
                t1v = t1[:].rearrange("p (c n) -> p c n", c=C)
                if blk < NBLK - 1:
                    nc.sync.dma_start(t1v, x[0:C, y0 + 1 : y0 + 129, :].transpose([1, 0, 2]))
                else:
                    nc.vector.memset(t1[:], 0.0)
                    nc.sync.dma_start(
                        t1[0:127, :].rearrange("p (c n) -> p c n", c=C),
                        x[0:C, y0 + 1 : y0 + 128, :].transpose([1, 0, 2]),
                    )
                vo = pmx.tile([128, C * W], f32, tag="vo")
                vov = vo[:].rearrange("p (c n) -> p c n", c=C)
                for c in range(C):
                    ta = pmx.tile([128, W - 1], f32, tag="ta")
                    tb = pmx.tile([128, W - 1], f32, tag="tb")
                    nc.vector.tensor_scalar(
                        ta[:], t0v[:, c, 0 : W - 1], wt[:, 0:1], None, Alu.mult
                    )
                    nc.vector.scalar_tensor_tensor(
                        tb[:], t0v[:, c, 1:W], wt[:, 1:2], ta[:], Alu.mult, Alu.add
                    )
                    nc.vector.scalar_tensor_tensor(
                        ta[:], t1v[:, c, 0 : W - 1], wt[:, 2:3], tb[:], Alu.mult, Alu.add
                    )
                    nc.vector.scalar_tensor_tensor(
                        vov[:, c, 0 : W - 1], t1v[:, c, 1:W], wt[:, 3:4], ta[:], Alu.mult, Alu.add
                    )
                    nc.vector.tensor_copy(vov[:, c, W - 1 : W], t0v[:, c, W - 1 : W])
                nc.sync.dma_start(v[0:C, y0 : y0 + 128, :].transpose([1, 0, 2]), vov)

        # ---- boundary lines of V: [left col, right col, top row, bottom row] ----
        with nc.allow_non_contiguous_dma(reason="column line extraction"):
            for c in range(C):
                nc.scalar.dma_start(ln[c, 0:H], v[c, 0:H, 0:1].rearrange("r o -> (r o)"))
                nc.scalar.dma_start(
                    ln[c, H : 2 * H], v[c, 0:H, W - 2 : W - 1].rearrange("r o -> (r o)")
                )
                nc.scalar.dma_start(ln[c, 2 * H : 3 * H], v[c, 0, :])
                nc.scalar.dma_start(ln[c, 3 * H : 4 * H], v[c, H - 2, :])

        # ---- reformat V into 8-row-interleaved V_p8: [rowgroup][col][parity] ----
        # A box fetch from V_p8 needs one contiguous 148*8*4B descriptor per
        # row-group instead of 8 x 592B row descriptors.
        with tc.tile_pool(name="rfmt", bufs=2) as rf:
            for c in range(C):
                rt = rf.tile([128, 8 * W], f32, tag="rt")
                nc.sync.dma_start(
                    rt[:], v[c, :, :].rearrange("(p a) b -> p (a b)", a=8)
                )
                it2 = rf.tile([128, 8 * W], f32, tag="it2")
                nc.vector.tensor_copy(
                    it2[:].rearrange("p (b a) -> p b a", a=8),
                    rt[:].rearrange("p (a b) -> p a b", a=8).transpose([0, 2, 1]),
                )
                nc.sync.dma_start(
                    vp8[c, :].rearrange("(p n) -> p n", p=128), it2[:]
                )

        # ---- gather waves ----
        gpool = ctx.enter_context(tc.tile_pool(name="gather", bufs=1))
        win = gpool.tile([128, NE], f32)
        nc.vector.memset(win[:], 0.0)
        for g in range(NG):
            d = DQUAD[g]
            nc.scalar.dma_start(win[16 * g + d : 16 * g + d + C, BOX:NE], ln[0:C, :])

        _engs3 = [nc.sync, nc.scalar, nc.gpsimd]
        box_engs = [_engs3[g % 3] for g in range(NG)]
        regs = [box_engs[g].alloc_register(f"boxoff{g}") for g in range(NG)]

        with tc.tile_pool(name="wave", bufs=2) as wpool:
            for wv in range(NWAVES):
                it = wpool.tile([128, NIDX // 16], i16, tag="it")
                nc.sync.dma_start(it[:], idxs[wv, :, :])
                for g in range(NG):
                    t = wv * NG + g
                    eng = box_engs[g]
                    eng.reg_load(regs[g], anct[0:1, t : t + 1])
                    src = bass.AP(
                        vp8.tensor, regs[g], [[H * W, C], [8 * W, NGRP8], [1, WWIN * 8]]
                    )
                    d = DQUAD[g]
                    eng.dma_start(
                        win[16 * g + d : 16 * g + d + C, 0:BOX].rearrange(
                            "p (r c2) -> p r c2", r=NGRP8
                        ),
                        src,
                    )
                go = wpool.tile([128, NIDX], f32, tag="go")
                nc.gpsimd.ap_gather(
                    go[:], win[:], it[:], channels=128, num_elems=NE, d=1, num_idxs=NIDX
                )
                for g in range(NG):
                    t = wv * NG + g
                    ty, tx = divmod(t, NT)
                    d = DQUAD[g]
                    oeng = nc.gpsimd
                    oeng.dma_start(
                        out[0:C, ty * TS : (ty + 1) * TS, tx * TS : (tx + 1) * TS],
                        go[16 * g + d : 16 * g + d + C, :].rearrange(
                            "p (a b) -> p a b", a=TS
                        ),
                    )

    nc.compile()
    return nc


def _plan(x_np, transform_np):
    """Host planner: bit-exact indices/weights (mirrors jax-CPU reference),
    per-core tile anchors, int16 gather streams, and host-patch values."""
    import jax
    import jax.numpy as jnp

    cpu = jax.devices("cpu")[0]
    with jax.default_device(cpu):
        transform = jnp.asarray(transform_np)
        A = transform[:, :4].reshape(B, 2, 2)
        t = transform[:, 4:6].reshape(B, 1, 2)
        Ainv = jnp.linalg.inv(A)
        t_inv = -jnp.matmul(t, Ainv)
        xg, yg = jnp.meshgrid(jnp.arange(W), jnp.arange(H), indexing="ij")
        pix = jnp.stack([xg.ravel(), yg.ravel()], axis=-1).astype(jnp.float32)
        out_pix = jnp.einsum("ni,bij->bnj", pix, Ainv) + t_inv
        c0 = jnp.clip(out_pix[..., 0], 0.0, H - 2)
        c1 = jnp.clip(out_pix[..., 1], 0.0, W - 2)
        i0 = c0.astype(jnp.int32)
        i1 = c1.astype(jnp.int32)
        dx0 = (c0 - i0.astype(jnp.float32))[:, 0]
        dy0 = (c1 - i1.astype(jnp.float32))[:, 0]
        i0 = np.asarray(i0)
        i1 = np.asarray(i1)
        dx0 = np.asarray(dx0)
        dy0 = np.asarray(dy0)

    plans = []
    for b in range(B):
        # row-major [y, x] index maps (k = x*H + y in reference order)
        I0 = np.ascontiguousarray(i0[b].reshape(W, H).T)
        I1 = np.ascontiguousarray(i1[b].reshape(W, H).T)
        w00 = np.float32((1 - dx0[b]) * (1 - dy0[b]))
        w10 = np.float32(dx0[b] * (1 - dy0[b]))
        w01 = np.float32((1 - dx0[b]) * dy0[b])
        w11 = np.float32(dx0[b] * dy0[b])
        wts = np.array([[w00, w10, w01, w11]], dtype=np.float32)

        # tiles [ty, tx, yl, xl]
        I0t = I0.reshape(NT, TS, NT, TS).transpose(0, 2, 1, 3)
        I1t = I1.reshape(NT, TS, NT, TS).transpose(0, 2, 1, 3)
        R0 = np.minimum(I1t.min(axis=(2, 3), keepdims=True), H - HWIN) & ~7
        C0 = np.minimum(I0t.min(axis=(2, 3), keepdims=True), W - WWIN)
        inbox = (
            (I1t >= R0)
            & (I1t < R0 + HWIN)
            & (I0t >= C0)
            & (I0t < C0 + WWIN)
        )
        dr = I1t - R0
        widx = (dr >> 3) * (WWIN * 8) + (I0t - C0) * 8 + (dr & 7)
        widx = np.where(inbox, widx, 0)
        lined = np.zeros_like(inbox)
        for cond, off, val in (
            (I0t == 0, BOX, I1t),
            (I0t == W - 2, BOX + H, I1t),
            (I1t == 0, BOX + 2 * H, I0t),
            (I1t == H - 2, BOX + 3 * H, I0t),
        ):
            use = cond & ~inbox & ~lined
            widx = np.where(use, off + val, widx)
            lined |= use
        covered = inbox | lined
        patch = ~covered

        anchors = (
            ((R0[:, :, 0, 0] >> 3) * (8 * W) + C0[:, :, 0, 0] * 8)
            .astype(np.int32)
            .reshape(1, NTILES)
        )
        # streams: tile t = wave*8 + g; stream pos j = yl*TS + xl;
        # wrapped: idxs[wave, 16g + j%16, j//16]
        wtile = widx.reshape(NTILES, NIDX).astype(np.int16)  # [t, j]
        wtile = wtile.reshape(NWAVES, NG, NIDX // 16, 16)  # [wave, g, s, j%16]
        streams = np.ascontiguousarray(
            wtile.transpose(0, 1, 3, 2).reshape(NWAVES, 128, NIDX // 16)
        )

        # host patch values (premixed blend, same f32 op order as device)
        py, px_ = np.nonzero(patch.transpose(0, 2, 1, 3).reshape(H, W))
        pv = None
        if len(py):
            r = I1.reshape(H, W)[py, px_]
            c = I0.reshape(H, W)[py, px_]
            xb = x_np[b]  # [C, H, W]
            pv = (
                (xb[:, r, c] * w00 + xb[:, r, c + 1] * w10)
                + xb[:, r + 1, c] * w01
            ) + xb[:, r + 1, c + 1] * w11  # [C, npatch]
        plans.append(
            dict(
                wts=wts,
                anc=anchors,
                idxs=streams,
                patch_yx=(py, px_),
                patch_vals=pv,
            )
        )
    return plans


def kernel(x, transform):
    """x: [8, 4, 1024, 1024] f32; transform: [8, 6] f32 -> [8, 4, 1024, 1024] f32."""
    from concourse.bass_utils import run_bass_kernel_spmd

    x = np.asarray(x, dtype=np.float32)
    transform = np.asarray(transform, dtype=np.float32)

    if "nc" not in _cache:
        _cache["nc"] = _build_program()
    nc = _cache["nc"]

    plans = _plan(x, transform)
    in_maps = []
    for b in range(B):
        p = plans[b]
        in_maps.append(
            {"x": x[b], "wts": p["wts"], "anc": p["anc"], "idxs": p["idxs"]}
        )
    res = run_bass_kernel_spmd(nc, in_maps, list(range(B)))
    outs = []
    for b in range(B):
        ob = res.results[b]["out"]
        py, px_ = plans[b]["patch_yx"]
        if len(py):
            ob = ob.copy()
            ob[:, py, px_] = plans[b]["patch_vals"]
        outs.append(ob)
    return np.stack(outs).astype(np.float32)

